# revision 36
# baseline (speedup 1.0000x reference)
"""Trainium2 Bass kernel: bidirectional conv-BN-relu message passing over H.

Reference semantics (per batch item, channels C, scan over H):
  forward:  new[0] = x[0];   new[h] = relu(bn(conv(new[h-1]))) + x[h]
  backward: out[H-1] = new[H-1]; out[h] = relu(bn(conv(out[h+1]))) + new[h]
conv = 1D conv along W, kernel 9, pad 4, C->C channels; BN (eval mode)
is a per-channel affine y*s + t.

Strategy: data-parallel over B across 8 cores (2 batch items per core).
Each conv step = 9 shifted-window matmuls accumulated in PSUM
(lhsT = per-tap [I,O] weights, rhs = padded state slice windows).
Matmul operands are bf16 (fp32 weights would disable fast-weight-load);
PSUM accumulation and the carry adds stay fp32 (~7e-3 relative error).

State-variable choice (the key scheduling trick): the BN scale s is
folded into the weights host-side (W' = s[o]*W), and the affine shift t
is folded into the STATES so each step's affine+relu+carry collapses to
ONE DVE scalar_tensor_tensor and the backward needs NO carry-prep op:
 - forward state  w(h) = new(h) + t, pad columns = t. Then
   conv(w-padded) = conv(new) + M t (M[o,i] = sum_k W'[o,i,k]) and
     w(h) = max(psum, q) + [x(h) + 2t - Mt],   q = Mt - t
   with the bracket baked into the shipped x slices (bf16, validated).
 - backward state v(h) = out(h) (true out-space), pad columns = 0,
   written to a SEPARATE tile (outT):
     v(h) = max(psum_v, -t) + w(h)
   The carry is the stored forward row itself — no ACT op, and writing
   v to its own tile keeps the backward reads of w free of WAR hazards
   against recent writes (tile dep tracking is coarse per-tile).
 - the forward h=63 step writes v(63) = w(63) - t directly into outT by
   shipping that one x slice with offset (t - Mt) instead of (2t - Mt).
Outputs leave in out-space: the host just slices off the pads.

Startup/drain engineering (trace-driven): engines cannot issue anything
before ~7.6us (framework preamble) and the first DMA packet lands
~1.45us after issue, so the startup is bandwidth-bound on the critical
bytes — x carries ship as bf16 (validated +2e-4 error), h=0 state ships
interior-only, weights lead the scalar queue in two chunks while state
and x ride sync. A burst of dummy matmuls (on a vector-memset tile)
holds the PE busy so the HAM clock-gate lifts (1.2 -> 2.4 GHz) before
the real stream. ONLY the sync and scalar DMA rings are used: the
gpsimd ring is ~5x slower and its exit DRAIN then gates the teardown
barrier. Outputs leave pad-inclusive (contiguous 528B*OB runs) on both
queues mid-stream; the final h=0 slices go out as four half-width
pieces, two per ring, so the drain transfers overlap.
"""

import os
from contextlib import ExitStack

import numpy as np
import ml_dtypes

import bass_rust
import concourse.bass as bass
import concourse.tile as tile
from concourse import mybir
from concourse.bass_utils import run_bass_kernel_spmd

B, C, H, W = 16, 128, 64, 256
K, PAD = 9, 4
NCORES = 8
BPC = B // NCORES  # batch items per core
WP = W + 2 * PAD
EPS = 1e-5
OB = 4  # output-slice DMA batch
NWARM = 42  # dummy N=64 matmuls bridging the PE from preamble-end to
# the arrival of the first weight/state DMAs (~53ns each at the cold
# 1.2GHz clock — fine granularity keeps the overshoot small while the
# run-to-run preamble/DMA jitter is ~±400ns)

F32 = mybir.dt.float32
BF16 = mybir.dt.bfloat16
NP_BF16 = ml_dtypes.bfloat16

_NC_CACHE: dict = {}
LAST_RESULTS = None  # stashed BassKernelResults for test.py introspection


def _xbounds(h_dim):
    """Input-batch spans for h>=1: small leading batches so the first conv
    steps aren't gated on a bulk transfer, then steady groups of 4."""
    bounds, sizes, lo = [], [1, 2, 4], 1
    while lo < h_dim:
        sz = sizes[0] if sizes else 4
        if sizes:
            sizes = sizes[1:]
        bounds.append((lo, min(lo + sz, h_dim)))
        lo += sz
    return bounds


def _build_nc(bpc=BPC, h_dim=H, w_dim=W):
    wp = w_dim + 2 * PAD
    nc = bass.Bass()
    x_d = nc.dram_tensor("x", [bpc, C, h_dim - 1, w_dim], BF16, kind="ExternalInput")
    # n0: [row-0 state pad-inclusive | q = Mt - t | -t | t | unused] per
    # channel — the per-channel scalars ride the state DMA in bf16 (a
    # separate [C,3] fp32 DMA costs 128 tiny ring-clogging packets).
    n0_d = nc.dram_tensor("n0", [bpc, C, wp + 4], BF16, kind="ExternalInput")
    w_d = nc.dram_tensor("w", [C, K, C], BF16, kind="ExternalInput")
    o_d = nc.dram_tensor("o", [bpc, C, h_dim, wp], BF16, kind="ExternalOutput")

    add = mybir.AluOpType.add
    mx = mybir.AluOpType.max

    xb_list = _xbounds(h_dim)
    xb_of_h = {}
    for i, (lo, hi) in enumerate(xb_list):
        for h in range(lo, hi):
            xb_of_h[h] = (i, lo, hi)

    with ExitStack() as ctx:
        tc = ctx.enter_context(tile.TileContext(nc))
        singles = ctx.enter_context(tc.tile_pool(name="singles", bufs=1))
        big = ctx.enter_context(tc.tile_pool(name="big", bufs=1))
        xs_pool = ctx.enter_context(tc.tile_pool(name="xs", bufs=6))
        pp = ctx.enter_context(tc.tile_pool(name="pp", bufs=8, space="PSUM"))

        # --- DMA ordering (sync + scalar rings only): sync: the h=0 state
        # rows (gate the first conv groups; params ride along), then the x
        # batches; scalar: weight taps 0-4 then 5-8 (tap 0 gates the first
        # matmul).
        new, outT, r0 = [], [], []
        for c in range(bpc):
            rt_ = singles.tile([C, wp + 4], BF16, tag=f"r0{c}", name=f"r0{c}")
            nc.sync.dma_start(out=rt_, in_=n0_d[c])
            r0.append(rt_)
            wtile = big.tile([C, h_dim, wp], BF16, tag=f"new{c}", name=f"new{c}")
            new.append(wtile)
            ot = big.tile([C, h_dim, wp], BF16, tag=f"out{c}", name=f"out{c}")
            outT.append(ot)
        # DVE/ACT scalar operands must be fp32: upconvert the bf16 packed
        # scalars once (placed after the startup memsets on the vector
        # FIFO so nothing early blocks on the n0 transfer).
        prt = singles.tile([C, 4], F32, tag="prt", name="prt")

        wt = singles.tile([C, K, C], BF16, tag="wt", name="wt")
        nc.scalar.dma_start(out=wt[:, 0:3, :], in_=w_d[:, 0:3, :])
        nc.scalar.dma_start(out=wt[:, 3:6, :], in_=w_d[:, 3:6, :])
        nc.scalar.dma_start(out=wt[:, 6:K, :], in_=w_d[:, 6:K, :])
        wr = [wt[:, k, :] for k in range(K)]

        # --- HAM warmup: dummy matmuls on a zeroed tile keep the PE busy
        # while the DMAs land, releasing the activity clock-gate.
        dummy = singles.tile([C, w_dim], BF16, tag="dummy", name="dummy")
        nc.vector.memset(dummy, 0.0)
        wm = pp.tile([C, w_dim], F32, tag="pt", name="wm", bufs=4)
        for _ in range(NWARM):
            nc.tensor.matmul(
                wm[:, 0:64], dummy[:, 0:C], dummy[:, 0:64],
                start=True, stop=True,
            )

        # Pad columns: w rows 1..62 hold t (row 0 pads ship inside n0; w row
        # 63 is never written or read — the h=63 result goes to outT);
        # v rows 0..63 hold 0. One 4D-AP op per chain covers both stripes.
        # The early forward groups wait on the w-pad fills (coarse tile
        # deps), so chain 0 rides the short vector startup chain and chain
        # 1 plus all the (late-needed) outT zero-fills go to the otherwise
        # idle Activation engine.
        nhr = h_dim - 2  # rows 1..62
        zp = singles.tile([C, h_dim, 2 * PAD], F32, tag="zp", name="zp")
        nc.vector.memset(zp, 0.0)
        nc.vector.tensor_copy(out=prt, in_=r0[0][:, wp : wp + 4])
        qt = prt[:, 0:1]
        nt = prt[:, 1:2]
        tt = prt[:, 2:3]

        def stripes(tile_h, row0, nrows):
            base = tile_h[:, row0 : row0 + nrows, 0:PAD]
            return bass.AP(
                base.tensor, base.offset,
                [base.ap[0], [wp, nrows], [PAD + w_dim, 2], [1, PAD]],
            )

        def zp4(nrows):
            base = zp[:, 0:nrows, 0:PAD]
            return bass.AP(
                base.tensor, base.offset,
                [base.ap[0], [2 * PAD, nrows], [PAD, 2], [1, PAD]],
            )

        IDENT = mybir.ActivationFunctionType.Identity
        nc.vector.tensor_scalar(
            out=stripes(new[0], 1, nhr), in0=zp4(nhr),
            scalar1=tt, scalar2=None, op0=add,
        )
        nc.scalar.activation(
            out=stripes(new[1], 1, nhr), in_=zp4(nhr),
            func=IDENT, bias=tt, scale=1.0,
        )
        for c in range(bpc):
            nc.scalar.activation(
                out=stripes(outT[c], 0, h_dim), in_=zp4(h_dim),
                func=IDENT, bias=0.0, scale=1.0,
            )

        def conv_group(src_row, pt):
            for k in range(K):
                nc.tensor.matmul(
                    pt,
                    wr[k],
                    src_row[:, k : k + w_dim],
                    start=(k == 0),
                    stop=(k == K - 1),
                )

        # Forward scan over H (both chains interleaved per h). h=63 writes
        # out-space directly into outT (its x slice ships with offset t-Mt).
        xtiles: list[dict[int, object]] = [dict() for _ in range(bpc)]
        for h in range(1, h_dim):
            bi, lo, hi = xb_of_h[h]
            if h == lo:
                for c in range(bpc):
                    xb = xs_pool.tile([C, 4, w_dim], BF16, tag="xb", name="xb")
                    nc.sync.dma_start(
                        out=xb[:, 0 : hi - lo, :], in_=x_d[c][:, lo - 1 : hi - 1, :]
                    )
                    xtiles[c][bi] = xb
            for c in range(bpc):
                pt = pp.tile([C, w_dim], F32, tag="pt", name="pt", bufs=4)
                src_row = r0[c][:, 0:wp] if h == 1 else new[c][:, h - 1, :]
                conv_group(src_row, pt)
                dst = outT[c] if h == h_dim - 1 else new[c]
                nc.vector.scalar_tensor_tensor(
                    out=dst[:, h, PAD : PAD + w_dim],
                    in0=pt,
                    scalar=qt,
                    in1=xtiles[c][bi][:, h - lo, :],
                    op0=mx,
                    op1=add,
                )

        # Backward scan: v(h) = max(conv(v(h+1)), -t) + w(h), written to
        # outT; slices stream out pad-inclusive in OB batches (contiguous
        # 528B*OB runs DMA much faster than pad-strided 512B packets).
        oq = [nc.scalar, nc.sync]
        for h in range(h_dim - 2, 0, -1):
            for c in range(bpc):
                pt = pp.tile([C, w_dim], F32, tag="pt", name="pt", bufs=4)
                conv_group(outT[c][:, h + 1, :], pt)
                nc.vector.scalar_tensor_tensor(
                    out=outT[c][:, h, PAD : PAD + w_dim],
                    in0=pt,
                    scalar=nt,
                    in1=new[c][:, h, PAD : PAD + w_dim],
                    op0=mx,
                    op1=add,
                )
            if h <= 3:
                # Ship the last rows singly as soon as each is final, so the
                # rings never build a trailing backlog that gates the drain.
                for c in range(bpc):
                    oq[c % 2].dma_start(
                        out=o_d[c][:, h : h + 1, :], in_=outT[c][:, h : h + 1, :]
                    )
            elif h % OB == 0:
                hi = min(h + OB, h_dim)
                for c in range(bpc):
                    oq[c % 2].dma_start(
                        out=o_d[c][:, h:hi, :], in_=outT[c][:, h:hi, :]
                    )

        # Final step (h=0) in two half-width PSUM groups per chain so the
        # very last DVE op and output transfers are half-sized — they sit
        # on the kernel's drain critical path. Four pieces, two per ring
        # (more, smaller pieces lose: each costs ~600ns of serial
        # engine-side descriptor-build on its ring).
        hw2 = w_dim // 2
        pieces = [(0, hw2, "pth"), (hw2, hw2, "pth")]
        for c in range(bpc):
            for pi, (lo, ln, ptag) in enumerate(pieces):
                pt = pp.tile([C, ln], F32, tag=ptag, name=ptag, bufs=2)
                for k in range(K):
                    nc.tensor.matmul(
                        pt,
                        wr[k],
                        outT[c][:, 1, lo + k : lo + k + ln],
                        start=(k == 0),
                        stop=(k == K - 1),
                    )
                nc.vector.scalar_tensor_tensor(
                    out=outT[c][:, 0, PAD + lo : PAD + lo + ln],
                    in0=pt,
                    scalar=nt,
                    in1=r0[c][:, PAD + lo : PAD + lo + ln],
                    op0=mx,
                    op1=add,
                )
                # Pad-inclusive pieces: the first carries the left pads,
                # the last the right pads.
                d0 = 0 if pi == 0 else PAD + lo
                d1 = wp if pi == len(pieces) - 1 else PAD + lo + ln
                oq[(c + pi) % 2].dma_start(
                    out=o_d[c][:, 0, d0:d1], in_=outT[c][:, 0, d0:d1]
                )

    # TRN2 caps most instructions at one semaphore wait (matmuls lower to an
    # LDWEIGHTS struct with a single wait slot); split any excess onto
    # EventSemaphore instructions like bacc does.
    bass_rust.generate_event_semaphores(nc)
    return nc


def _get_nc():
    key = (BPC, H, W)
    if key not in _NC_CACHE:
        _NC_CACHE[key] = _build_nc()
    return _NC_CACHE[key]


def _prep_params(conv_w, gamma, beta, run_mean, run_var):
    """Fold BN scale into the weights; build the t-shift constants.

    Returns (w_t [I,K,O] bf16 with s folded, pr [C,3] f32, t, Mt [C] f64).
    """
    s = gamma.astype(np.float64) / np.sqrt(run_var.astype(np.float64) + EPS)
    t = beta.astype(np.float64) - run_mean.astype(np.float64) * s
    w_s = s[:, None, None] * conv_w.astype(np.float64)  # [O,I,K]
    w_t = np.ascontiguousarray(w_s.transpose(1, 2, 0)).astype(NP_BF16)

    # Mt from the SHIPPED (bf16-rounded) weights and the bf16-rounded t
    # that actually sits in the pad columns, so the algebra matches the
    # device bit-for-bit up to fp32 accumulation.
    w64 = w_t.astype(np.float64)  # [I,K,O]
    tb = t.astype(np.float32).astype(NP_BF16).astype(np.float64)
    mt = np.einsum("iko,i->o", w64, tb)  # [O]
    q = mt - t
    pr = np.stack([q, -t, t, np.zeros(C)], axis=1)
    return w_t, pr, t, mt


def kernel(inputs, conv_w, gamma, beta, run_mean, run_var):
    global LAST_RESULTS
    conv_w, gamma, beta, run_mean, run_var = (
        np.asarray(a) for a in (conv_w, gamma, beta, run_mean, run_var)
    )
    w_t, pr, t, mt = _prep_params(conv_w, gamma, beta, run_mean, run_var)
    x = np.asarray(inputs, dtype=np.float64)  # [B,C,H,W]
    # h=0 state interior: w(0) = x[:,:,0] + t. x carries for h=1..62 ship
    # with offset (2t - Mt); the h=63 slice with (t - Mt) so that step
    # lands directly in out-space.
    # Row-0 state ships pad-inclusive: pads = t, interior = x[:,:,0] + t,
    # followed by the per-channel scalars [q, -t, t, 0].
    n0 = np.empty((B, C, WP + 4), np.float64)
    n0[:, :, 0:WP] = t[None, :, None]
    n0[:, :, PAD : PAD + W] = x[:, :, 0] + t[None, :, None]
    n0[:, :, WP : WP + 4] = pr[None, :, :]
    n0 = n0.astype(NP_BF16)
    off = (2.0 * t - mt)[None, :, None, None]
    xq = x[:, :, 1:, :] + off
    xq[:, :, -1, :] -= t[None, :, None]
    xq = np.ascontiguousarray(xq).astype(NP_BF16)
    in_maps = [
        dict(
            x=xq[c * BPC : (c + 1) * BPC],
            n0=n0[c * BPC : (c + 1) * BPC],
            w=w_t,
            pr=pr,
        )
        for c in range(NCORES)
    ]
    nc = _get_nc()
    trace = os.environ.get("KERNEL_TRACE", "0") == "1"
    res = run_bass_kernel_spmd(
        nc, in_maps, core_ids=list(range(NCORES)), trace=trace
    )
    LAST_RESULTS = res
    out = np.concatenate(
        [np.asarray(res.results[c]["o"]) for c in range(NCORES)], axis=0
    )[:, :, :, PAD : PAD + W].astype(np.float32)
    return out  # already in out-space


# revision 37
# speedup vs baseline: 1.1966x; 1.1966x over previous
"""Trainium2 Bass kernel: bidirectional conv-BN-relu message passing over H.

Reference semantics (per batch item, channels C, scan over H):
  forward:  new[0] = x[0];   new[h] = relu(bn(conv(new[h-1]))) + x[h]
  backward: out[H-1] = new[H-1]; out[h] = relu(bn(conv(out[h+1]))) + new[h]
conv = 1D conv along W, kernel 9, pad 4, C->C channels; BN (eval mode)
is a per-channel affine y*s + t.

Strategy: data-parallel over B across 8 cores (2 batch items per core).
Each conv step = 9 shifted-window matmuls accumulated in PSUM
(lhsT = per-tap [I,O] weights, rhs = padded state slice windows).
Matmul operands are bf16 (fp32 weights would disable fast-weight-load);
PSUM accumulation and the carry adds stay fp32 (~7e-3 relative error).

State-variable choice (the key scheduling trick): the BN scale s is
folded into the weights host-side (W' = s[o]*W), and the affine shift t
is folded into the STATES so each step's affine+relu+carry collapses to
ONE DVE scalar_tensor_tensor and the backward needs NO carry-prep op:
 - forward state  w(h) = new(h) + t, pad columns = t. Then
   conv(w-padded) = conv(new) + M t (M[o,i] = sum_k W'[o,i,k]) and
     w(h) = max(psum, q) + [x(h) + 2t - Mt],   q = Mt - t
   with the bracket baked into the shipped x slices (bf16, validated).
 - backward state v(h) = out(h) (true out-space), pad columns = 0,
   written to a SEPARATE tile (outT):
     v(h) = max(psum_v, -t) + w(h)
   The carry is the stored forward row itself — no ACT op, and writing
   v to its own tile keeps the backward reads of w free of WAR hazards
   against recent writes (tile dep tracking is coarse per-tile).
 - the forward h=63 step writes v(63) = w(63) - t directly into outT by
   shipping that one x slice with offset (t - Mt) instead of (2t - Mt).
Outputs leave in out-space: the host just slices off the pads.

Startup/drain engineering (trace-driven): engines cannot issue anything
before ~7.6us (framework preamble) and the first DMA packet lands
~1.45us after issue, so the startup is bandwidth-bound on the critical
bytes — x carries ship as bf16 (validated +2e-4 error), h=0 state ships
interior-only, weights lead the scalar queue in two chunks while state
and x ride sync. A burst of dummy matmuls (on a vector-memset tile)
holds the PE busy so the HAM clock-gate lifts (1.2 -> 2.4 GHz) before
the real stream. ONLY the sync and scalar DMA rings are used: the
gpsimd ring is ~5x slower and its exit DRAIN then gates the teardown
barrier. Outputs leave pad-inclusive (contiguous 528B*OB runs) on both
queues mid-stream; the final h=0 slices go out as four half-width
pieces, two per ring, so the drain transfers overlap.
"""

import os
from contextlib import ExitStack

import numpy as np
import ml_dtypes

import bass_rust
import concourse.bass as bass
import concourse.tile as tile
from concourse import mybir
from concourse.bass_utils import run_bass_kernel_spmd

B, C, H, W = 16, 128, 64, 256
K, PAD = 9, 4
NCORES = 8
BPC = B // NCORES  # batch items per core
WP = W + 2 * PAD
EPS = 1e-5
OB = 4  # output-slice DMA batch
NWARM = 42  # dummy N=64 matmuls bridging the PE from preamble-end to
# the arrival of the first weight/state DMAs (~53ns each at the cold
# 1.2GHz clock — fine granularity keeps the overshoot small while the
# run-to-run preamble/DMA jitter is ~±400ns)

F32 = mybir.dt.float32
BF16 = mybir.dt.bfloat16
NP_BF16 = ml_dtypes.bfloat16

_NC_CACHE: dict = {}
LAST_RESULTS = None  # stashed BassKernelResults for test.py introspection


def _xbounds(h_dim):
    """Input-batch spans for h>=1: small leading batches so the first conv
    steps aren't gated on a bulk transfer, then steady groups of 4."""
    bounds, sizes, lo = [], [1, 2, 4], 1
    while lo < h_dim:
        sz = sizes[0] if sizes else 4
        if sizes:
            sizes = sizes[1:]
        bounds.append((lo, min(lo + sz, h_dim)))
        lo += sz
    return bounds


def _build_nc(bpc=BPC, h_dim=H, w_dim=W):
    wp = w_dim + 2 * PAD
    nc = bass.Bass()
    x_d = nc.dram_tensor("x", [bpc, C, h_dim - 1, w_dim], BF16, kind="ExternalInput")
    # n0: [row-0 state pad-inclusive | q = Mt - t | -t | t | unused] per
    # channel — the per-channel scalars ride the state DMA in bf16 (a
    # separate [C,3] fp32 DMA costs 128 tiny ring-clogging packets).
    n0_d = nc.dram_tensor("n0", [bpc, C, wp + 4], BF16, kind="ExternalInput")
    w_d = nc.dram_tensor("w", [C, K, C], BF16, kind="ExternalInput")
    o_d = nc.dram_tensor("o", [bpc, C, h_dim, wp], BF16, kind="ExternalOutput")

    add = mybir.AluOpType.add
    mx = mybir.AluOpType.max

    xb_list = _xbounds(h_dim)
    xb_of_h = {}
    for i, (lo, hi) in enumerate(xb_list):
        for h in range(lo, hi):
            xb_of_h[h] = (i, lo, hi)

    with ExitStack() as ctx:
        tc = ctx.enter_context(tile.TileContext(nc))
        singles = ctx.enter_context(tc.tile_pool(name="singles", bufs=1))
        big = ctx.enter_context(tc.tile_pool(name="big", bufs=1))
        xs_pool = ctx.enter_context(tc.tile_pool(name="xs", bufs=6))
        pp = ctx.enter_context(tc.tile_pool(name="pp", bufs=8, space="PSUM"))

        # --- DMA ordering (sync + scalar rings only): sync: the h=0 state
        # rows (gate the first conv groups; params ride along), then the x
        # batches; scalar: weight taps 0-4 then 5-8 (tap 0 gates the first
        # matmul).
        new, outT, r0 = [], [], []
        for c in range(bpc):
            rt_ = singles.tile([C, wp + 4], BF16, tag=f"r0{c}", name=f"r0{c}")
            nc.sync.dma_start(out=rt_, in_=n0_d[c])
            r0.append(rt_)
            wtile = big.tile([C, h_dim, wp], BF16, tag=f"new{c}", name=f"new{c}")
            new.append(wtile)
            ot = big.tile([C, h_dim, wp], BF16, tag=f"out{c}", name=f"out{c}")
            outT.append(ot)
        # DVE/ACT scalar operands must be fp32: upconvert the bf16 packed
        # scalars once (placed after the startup memsets on the vector
        # FIFO so nothing early blocks on the n0 transfer).
        prt = singles.tile([C, 4], F32, tag="prt", name="prt")

        wt = singles.tile([C, K, C], BF16, tag="wt", name="wt")
        nc.scalar.dma_start(out=wt[:, 0:3, :], in_=w_d[:, 0:3, :])
        nc.scalar.dma_start(out=wt[:, 3:6, :], in_=w_d[:, 3:6, :])
        nc.scalar.dma_start(out=wt[:, 6:K, :], in_=w_d[:, 6:K, :])
        wr = [wt[:, k, :] for k in range(K)]

        # --- HAM warmup: dummy matmuls on a zeroed tile keep the PE busy
        # while the DMAs land, releasing the activity clock-gate.
        dummy = singles.tile([C, w_dim], BF16, tag="dummy", name="dummy")
        nc.vector.memset(dummy, 0.0)
        wm = pp.tile([C, w_dim], F32, tag="pt", name="wm", bufs=4)
        for _ in range(NWARM):
            nc.tensor.matmul(
                wm[:, 0:64], dummy[:, 0:C], dummy[:, 0:64],
                start=True, stop=True,
            )

        # Pad columns: w rows 1..62 hold t (row 0 pads ship inside n0; w row
        # 63 is never written or read — the h=63 result goes to outT);
        # v rows 0..63 hold 0. One 4D-AP op per chain covers both stripes.
        # The early forward groups wait on the w-pad fills (coarse tile
        # deps), so chain 0 rides the short vector startup chain and chain
        # 1 plus all the (late-needed) outT zero-fills go to the otherwise
        # idle Activation engine.
        nhr = h_dim - 2  # rows 1..62
        zp = singles.tile([C, h_dim, 2 * PAD], F32, tag="zp", name="zp")
        nc.vector.memset(zp, 0.0)
        nc.vector.tensor_copy(out=prt, in_=r0[0][:, wp : wp + 4])
        qt = prt[:, 0:1]
        nt = prt[:, 1:2]
        tt = prt[:, 2:3]

        def stripes(tile_h, row0, nrows):
            base = tile_h[:, row0 : row0 + nrows, 0:PAD]
            return bass.AP(
                base.tensor, base.offset,
                [base.ap[0], [wp, nrows], [PAD + w_dim, 2], [1, PAD]],
            )

        def zp4(nrows):
            base = zp[:, 0:nrows, 0:PAD]
            return bass.AP(
                base.tensor, base.offset,
                [base.ap[0], [2 * PAD, nrows], [PAD, 2], [1, PAD]],
            )

        IDENT = mybir.ActivationFunctionType.Identity
        nc.vector.tensor_scalar(
            out=stripes(new[0], 1, nhr), in0=zp4(nhr),
            scalar1=tt, scalar2=None, op0=add,
        )
        nc.scalar.activation(
            out=stripes(new[1], 1, nhr), in_=zp4(nhr),
            func=IDENT, bias=tt, scale=1.0,
        )
        for c in range(bpc):
            nc.scalar.activation(
                out=stripes(outT[c], 0, h_dim), in_=zp4(h_dim),
                func=IDENT, bias=0.0, scale=1.0,
            )

        def conv_group(src_row, pt):
            for k in range(K):
                nc.tensor.matmul(
                    pt,
                    wr[k],
                    src_row[:, k : k + w_dim],
                    start=(k == 0),
                    stop=(k == K - 1),
                )

        # Forward scan over H (both chains interleaved per h). h=63 writes
        # out-space directly into outT (its x slice ships with offset t-Mt).
        xtiles: list[dict[int, object]] = [dict() for _ in range(bpc)]
        for h in range(1, h_dim):
            bi, lo, hi = xb_of_h[h]
            if h == lo:
                for c in range(bpc):
                    xb = xs_pool.tile([C, 4, w_dim], BF16, tag="xb", name="xb")
                    nc.sync.dma_start(
                        out=xb[:, 0 : hi - lo, :], in_=x_d[c][:, lo - 1 : hi - 1, :]
                    )
                    xtiles[c][bi] = xb
            for c in range(bpc):
                pt = pp.tile([C, w_dim], F32, tag="pt", name="pt", bufs=4)
                src_row = r0[c][:, 0:wp] if h == 1 else new[c][:, h - 1, :]
                conv_group(src_row, pt)
                dst = outT[c] if h == h_dim - 1 else new[c]
                nc.vector.scalar_tensor_tensor(
                    out=dst[:, h, PAD : PAD + w_dim],
                    in0=pt,
                    scalar=qt,
                    in1=xtiles[c][bi][:, h - lo, :],
                    op0=mx,
                    op1=add,
                )

        # Backward scan: v(h) = max(conv(v(h+1)), -t) + w(h), written to
        # outT; slices stream out pad-inclusive in OB batches (contiguous
        # 528B*OB runs DMA much faster than pad-strided 512B packets).
        oq = [nc.scalar, nc.sync]
        for h in range(h_dim - 2, 0, -1):
            for c in range(bpc):
                pt = pp.tile([C, w_dim], F32, tag="pt", name="pt", bufs=4)
                conv_group(outT[c][:, h + 1, :], pt)
                nc.vector.scalar_tensor_tensor(
                    out=outT[c][:, h, PAD : PAD + w_dim],
                    in0=pt,
                    scalar=nt,
                    in1=new[c][:, h, PAD : PAD + w_dim],
                    op0=mx,
                    op1=add,
                )
            if h == 2:
                # Split the final OB-batch so the very last transfers (which
                # gate the drain) are small.
                for c in range(bpc):
                    oq[c % 2].dma_start(
                        out=o_d[c][:, 2:4, :], in_=outT[c][:, 2:4, :]
                    )
            elif h == 1:
                for c in range(bpc):
                    oq[c % 2].dma_start(
                        out=o_d[c][:, 1:2, :], in_=outT[c][:, 1:2, :]
                    )
            elif h % OB == 0:
                hi = min(h + OB, h_dim)
                for c in range(bpc):
                    oq[c % 2].dma_start(
                        out=o_d[c][:, h:hi, :], in_=outT[c][:, h:hi, :]
                    )

        # Final step (h=0) in two half-width PSUM groups per chain so the
        # very last DVE op and output transfers are half-sized — they sit
        # on the kernel's drain critical path. Four pieces, two per ring
        # (more, smaller pieces lose: each costs ~600ns of serial
        # engine-side descriptor-build on its ring).
        hw2 = w_dim // 2
        pieces = [(0, hw2, "pth"), (hw2, hw2, "pth")]
        for c in range(bpc):
            for pi, (lo, ln, ptag) in enumerate(pieces):
                pt = pp.tile([C, ln], F32, tag=ptag, name=ptag, bufs=2)
                for k in range(K):
                    nc.tensor.matmul(
                        pt,
                        wr[k],
                        outT[c][:, 1, lo + k : lo + k + ln],
                        start=(k == 0),
                        stop=(k == K - 1),
                    )
                nc.vector.scalar_tensor_tensor(
                    out=outT[c][:, 0, PAD + lo : PAD + lo + ln],
                    in0=pt,
                    scalar=nt,
                    in1=r0[c][:, PAD + lo : PAD + lo + ln],
                    op0=mx,
                    op1=add,
                )
                # Pad-inclusive pieces: the first carries the left pads,
                # the last the right pads.
                d0 = 0 if pi == 0 else PAD + lo
                d1 = wp if pi == len(pieces) - 1 else PAD + lo + ln
                oq[(c + pi) % 2].dma_start(
                    out=o_d[c][:, 0, d0:d1], in_=outT[c][:, 0, d0:d1]
                )

    # TRN2 caps most instructions at one semaphore wait (matmuls lower to an
    # LDWEIGHTS struct with a single wait slot); split any excess onto
    # EventSemaphore instructions like bacc does.
    bass_rust.generate_event_semaphores(nc)
    return nc


def _get_nc():
    key = (BPC, H, W)
    if key not in _NC_CACHE:
        _NC_CACHE[key] = _build_nc()
    return _NC_CACHE[key]


def _prep_params(conv_w, gamma, beta, run_mean, run_var):
    """Fold BN scale into the weights; build the t-shift constants.

    Returns (w_t [I,K,O] bf16 with s folded, pr [C,3] f32, t, Mt [C] f64).
    """
    s = gamma.astype(np.float64) / np.sqrt(run_var.astype(np.float64) + EPS)
    t = beta.astype(np.float64) - run_mean.astype(np.float64) * s
    w_s = s[:, None, None] * conv_w.astype(np.float64)  # [O,I,K]
    w_t = np.ascontiguousarray(w_s.transpose(1, 2, 0)).astype(NP_BF16)

    # Mt from the SHIPPED (bf16-rounded) weights and the bf16-rounded t
    # that actually sits in the pad columns, so the algebra matches the
    # device bit-for-bit up to fp32 accumulation.
    w64 = w_t.astype(np.float64)  # [I,K,O]
    tb = t.astype(np.float32).astype(NP_BF16).astype(np.float64)
    mt = np.einsum("iko,i->o", w64, tb)  # [O]
    q = mt - t
    pr = np.stack([q, -t, t, np.zeros(C)], axis=1)
    return w_t, pr, t, mt


def kernel(inputs, conv_w, gamma, beta, run_mean, run_var):
    global LAST_RESULTS
    conv_w, gamma, beta, run_mean, run_var = (
        np.asarray(a) for a in (conv_w, gamma, beta, run_mean, run_var)
    )
    w_t, pr, t, mt = _prep_params(conv_w, gamma, beta, run_mean, run_var)
    x = np.asarray(inputs, dtype=np.float64)  # [B,C,H,W]
    # h=0 state interior: w(0) = x[:,:,0] + t. x carries for h=1..62 ship
    # with offset (2t - Mt); the h=63 slice with (t - Mt) so that step
    # lands directly in out-space.
    # Row-0 state ships pad-inclusive: pads = t, interior = x[:,:,0] + t,
    # followed by the per-channel scalars [q, -t, t, 0].
    n0 = np.empty((B, C, WP + 4), np.float64)
    n0[:, :, 0:WP] = t[None, :, None]
    n0[:, :, PAD : PAD + W] = x[:, :, 0] + t[None, :, None]
    n0[:, :, WP : WP + 4] = pr[None, :, :]
    n0 = n0.astype(NP_BF16)
    off = (2.0 * t - mt)[None, :, None, None]
    xq = x[:, :, 1:, :] + off
    xq[:, :, -1, :] -= t[None, :, None]
    xq = np.ascontiguousarray(xq).astype(NP_BF16)
    in_maps = [
        dict(
            x=xq[c * BPC : (c + 1) * BPC],
            n0=n0[c * BPC : (c + 1) * BPC],
            w=w_t,
            pr=pr,
        )
        for c in range(NCORES)
    ]
    nc = _get_nc()
    trace = os.environ.get("KERNEL_TRACE", "0") == "1"
    res = run_bass_kernel_spmd(
        nc, in_maps, core_ids=list(range(NCORES)), trace=trace
    )
    LAST_RESULTS = res
    out = np.concatenate(
        [np.asarray(res.results[c]["o"]) for c in range(NCORES)], axis=0
    )[:, :, :, PAD : PAD + W].astype(np.float32)
    return out  # already in out-space
